# revision 1
# baseline (speedup 1.0000x reference)
"""GCN (2x GCNConv + linear head) Trainium2 kernel, 8-core graph-parallel.

Strategy
--------
Nodes are sharded across 8 NeuronCores (dst-range ownership). Per layer:
  h_pre = a_prev @ W          (TensorE, per 128-node tile)
  g     = dinv * h_pre        (fused into PSUM eviction; fp16 copy for comms)
  AllGather g -> full table   (collective, fp16, packed [N,64])
  aggregation: for each edge (s,d): acc[d] += g[s]
      - gather g[s] via dma_gather (GPSIMD SWDGE): table viewed as
        [quads, 256] fp16 (4 nodes / 512B per row, int16 quad indices)
      - segmented reduction via one-hot matmul: per 128-edge tile,
        B[e, j] = (dst_local[e] % 128 == j) built with a DVE is_equal;
        PSUM accumulates all tiles of a 128-dst window. Race-free (unlike
        dma_scatter_add, whose CCE read-modify-write loses updates for
        duplicate indices).
  out = relu(dinv * acc + dinv*g_self + b)   (folded: g'' = dinv*g + b)
  transpose to feature-major for the next layer's matmul (TensorE).
Head: out2 @ Wc + bc per 512-node chunk; host reassembles [N, 40].

The edge schedule (tiles per (window, src%4) cell) is data-dependent but
shared across cores: per-cell tile counts are the max over cores, with
dummy edges (gather idx 0, dst one-hot row of zeros) padding each cell.
"""
import sys

sys.path.insert(0, "/opt/trn_rl_repo")

import numpy as np

NCORES = 8
WCH = 6  # dst windows (128 nodes each) per gather chunk


def _div_up(a, b):
    return -(-a // b)


def _wrap_idx16(idx):
    """Per-call idx layout: element i -> partition i%16, slot i//16."""
    assert len(idx) % 16 == 0
    return idx.astype(np.int16).reshape(-1, 16).T  # [16, S/16]


def _schedule(edge_index, N):
    """Build the shared tile schedule + per-core gather/one-hot data."""
    NPC = N // NCORES
    SLOTS = _div_up(NPC, 128)
    NPCP = SLOTS * 128
    assert NPCP % 4 == 0

    src = np.asarray(edge_index[0], np.int64)
    dst = np.asarray(edge_index[1], np.int64)
    deg = np.bincount(dst, minlength=N).astype(np.float32) + 1.0
    dinv = 1.0 / np.sqrt(deg)

    owner = src // NPC
    sp = owner * NPCP + (src - owner * NPC)  # padded global src index
    quad = (sp // 4).astype(np.int64)
    qq = (sp % 4).astype(np.int64)
    assert quad.max() < 32768, "quad index must fit int16"

    NW = SLOTS  # windows of 128 dst nodes
    core_data = []
    counts = np.zeros((NCORES, NW, 4), np.int64)
    for c in range(NCORES):
        m = (dst >= c * NPC) & (dst < (c + 1) * NPC)
        dl = dst[m] - c * NPC
        w = dl // 128
        key = w * 4 + qq[m]
        order = np.argsort(key, kind="stable")
        core_data.append((quad[m][order], (dl % 128)[order], key[order]))
        np.add.at(counts[c], (w, qq[m]), 1)

    T_cell = _div_up(counts, 128).max(axis=0)  # [NW, 4] shared tiles/cell

    # tile order: chunks of WCH windows; within chunk: q-major (one gather
    # call per q, reading table quad-rows at byte offset 128*q), then w.
    # A window's psum accumulates across its 4 q-cells within one chunk.
    chunks = []  # (S, idx_off16, t_off, [(tl, q, w, start, stop)], qsizes)
    t_off = 0
    idx_off = 0
    for w0 in range(0, NW, WCH):
        ws = range(w0, min(w0 + WCH, NW))
        tiles = []
        first = {w: True for w in ws}
        ntiles_left = {w: int(T_cell[w].sum()) for w in ws}
        qsizes = []
        for q in range(4):
            qn = 0
            for w in ws:
                for _ in range(T_cell[w, q]):
                    st = first[w]
                    first[w] = False
                    ntiles_left[w] -= 1
                    sp = ntiles_left[w] == 0
                    tiles.append((len(tiles), q, w, st, sp))
                    qn += 1
            qsizes.append(qn * 128)
        S = len(tiles) * 128
        chunks.append((S, idx_off // 16, t_off, tiles, qsizes))
        t_off += len(tiles)
        idx_off += S
    T_total = t_off
    S_total = idx_off

    # per-core arrays in tile order
    gidx_cores, dmod_cores = [], []
    for c in range(NCORES):
        cq, cdmod, ckey = core_data[c]
        bounds = np.searchsorted(ckey, np.arange(NW * 4 + 1))
        gidx = np.zeros(S_total, np.int16)
        dmod = np.full(S_total, -1.0, np.float16)
        pos = 0
        for _S, _off16, _toff, tiles, _qs in chunks:
            cell_seen = set()
            for tl, q, w, _st, _sp in tiles:
                k = (w, q)
                if k not in cell_seen:
                    cell_seen.add(k)
                    lo, hi = bounds[w * 4 + q], bounds[w * 4 + q + 1]
                    n = hi - lo
                    cap = T_cell[w, q] * 128
                    gidx[pos : pos + n] = cq[lo:hi]
                    dmod[pos : pos + n] = cdmod[lo:hi].astype(np.float16)
                    pos += cap
        # wrap per gather call (one per nonzero q section of each chunk)
        wrapped = []
        p = 0
        for _S, _off16, _toff, _tiles, qs in chunks:
            for qn in qs:
                if qn:
                    wrapped.append(_wrap_idx16(gidx[p : p + qn]))
                    p += qn
        gw = np.concatenate(wrapped, axis=1)  # [16, S_total/16]
        gidx_cores.append(np.tile(gw, (8, 1)).copy())
        dmod_cores.append(dmod.reshape(T_total, 128).T.copy())  # [128, T]

    sched = dict(
        NPC=NPC, SLOTS=SLOTS, NPCP=NPCP, NW=NW, N=N,
        chunks=chunks, T_total=T_total, S_total=S_total,
    )
    return sched, dinv, gidx_cores, dmod_cores


def _patch_dma_gather_assert():
    """Relax bass's elem_size_bytes %256 assert to %128 for dma_gather.

    The 256B granularity is a transpose-path restriction; the non-transpose
    ucode handles 128B payloads with a 256B-multiple stride (verified exact
    on hardware). Gathering 128B per edge instead of 512B halves DMA time.
    """
    import inspect
    import textwrap
    import concourse.bass as bass

    needle = "elem_size_bytes > 0 and elem_size_bytes % 256 == 0"
    src = inspect.getsource(bass.BassGpSimd.dma_gather)
    if needle not in src:
        return
    src = textwrap.dedent(src.replace(
        needle, "elem_size_bytes > 0 and elem_size_bytes % 128 == 0"))
    ns = {}
    exec(compile(src, "<dma_gather_patched>", "exec"), bass.__dict__, ns)
    bass.BassGpSimd.dma_gather = ns["dma_gather"]


def _build(sched, F, H, C):
    import os
    import concourse.bacc as bacc
    import concourse.mybir as mybir
    import concourse.tile as tile

    SKIP_AG = os.environ.get("SKIP_AG", "0") == "1"
    SKIP_GATHER = os.environ.get("SKIP_GATHER", "0") == "1"
    SKIP_AGG = os.environ.get("SKIP_AGG", "0") == "1"
    SKIP_TPOSE = os.environ.get("SKIP_TPOSE", "0") == "1"
    STOPAT = int(os.environ.get("STOPAT", "99"))

    f32 = mybir.dt.float32
    f16 = mybir.dt.float16
    i16 = mybir.dt.int16
    Relu = mybir.ActivationFunctionType.Relu
    SLOTS, NPCP = sched["SLOTS"], sched["NPCP"]
    S_total, T_total = sched["S_total"], sched["T_total"]
    chunks = sched["chunks"]
    TBL = NCORES * NPCP

    _patch_dma_gather_assert()
    nc = bacc.Bacc("TRN2", target_bir_lowering=False, debug=False,
                   num_devices=NCORES)

    x_t = nc.dram_tensor("x_t", [F, NPCP], f32, kind="ExternalInput")
    W1 = nc.dram_tensor("W1", [F, H], f32, kind="ExternalInput")
    W2h = nc.dram_tensor("W2h", [H, H], f16, kind="ExternalInput")
    Wch = nc.dram_tensor("Wch", [H, C], f16, kind="ExternalInput")
    b1in = nc.dram_tensor("b1r", [128, H], f32, kind="ExternalInput")
    b2in = nc.dram_tensor("b2r", [128, H], f32, kind="ExternalInput")
    bcin = nc.dram_tensor("bcr", [C, 1], f32, kind="ExternalInput")
    dinvin = nc.dram_tensor("dinv_nm", [128, SLOTS], f32, kind="ExternalInput")
    iotain = nc.dram_tensor("iota2d", [128, 128], f16, kind="ExternalInput")
    identin = nc.dram_tensor("ident16", [128, 128], f16, kind="ExternalInput")
    gidxin = nc.dram_tensor("gidx", [128, S_total // 16], i16, kind="ExternalInput")
    dmodin = nc.dram_tensor("dmod", [128, T_total], f16, kind="ExternalInput")
    out_d = nc.dram_tensor("out", [C, NPCP], f32, kind="ExternalOutput")

    with tile.TileContext(nc) as tc:
        with (
            tc.tile_pool(name="const", bufs=1) as cp,
            tc.tile_pool(name="big", bufs=1) as bigp,
            tc.tile_pool(name="xt", bufs=3) as xtp,
            tc.tile_pool(name="msgs", bufs=2) as msgp,
            tc.tile_pool(name="bmat", bufs=2) as bp,
            tc.tile_pool(name="tmp", bufs=4) as tmpp,
            tc.tile_pool(name="outsb", bufs=3) as outp,
            tc.tile_pool(name="pwin", bufs=6, space="PSUM") as pwin,
            tc.tile_pool(name="pmisc", bufs=2, space="PSUM") as pmisc,
            tc.tile_pool(name="dram", bufs=4, space="DRAM") as dram,
        ):
            def load_const(dt, ten, shape):
                t = cp.tile(shape, dt, tag=ten.name)
                nc.sync.dma_start(out=t[:], in_=ten[:])
                return t

            W1s = load_const(f32, W1, [F, H])
            W2s = load_const(f16, W2h, [H, H])
            Wcs = load_const(f16, Wch, [H, C])
            b1s = load_const(f32, b1in, [128, H])
            b2s = load_const(f32, b2in, [128, H])
            bcs = load_const(f32, bcin, [C, 1])
            dinvs = load_const(f32, dinvin, [128, SLOTS])
            iotas = load_const(f16, iotain, [128, 128])
            idents = load_const(f16, identin, [128, 128])
            gidxs = load_const(i16, gidxin, [128, S_total // 16])
            dmods = load_const(f16, dmodin, [128, T_total])

            def aggregate(table_ap, g32, bias2d, out16):
                """acc = sum_edges g[src]; out16 = relu-input fp16."""
                # fold: g'' = dinv*g + b  (self-loop + bias), in place
                nc.vector.tensor_tensor(
                    out=g32[:], in0=g32[:],
                    in1=dinvs[:].unsqueeze(2).to_broadcast([128, SLOTS, H]),
                    op=mybir.AluOpType.mult)
                nc.vector.tensor_tensor(
                    out=g32[:], in0=g32[:],
                    in1=bias2d[:].unsqueeze(1).to_broadcast([128, SLOTS, H]),
                    op=mybir.AluOpType.add)
                if SKIP_AGG:
                    for w in range(SLOTS):
                        nc.vector.tensor_copy(out=out16[:, w, :], in_=g32[:, w, :])
                    nc.scalar.activation(
                        out16[:].rearrange("p s d -> p (s d)"),
                        out16[:].rearrange("p s d -> p (s d)"), Relu)
                    return
                tquads = table_ap.rearrange("(a b) d -> a (b d)", b=4)
                for S, off16, toff, tiles, qsizes in chunks:
                    T = S // 128
                    msgs = msgp.tile([128, T, H], f16, tag="msgs")
                    o16 = off16
                    t0 = 0
                    for q in range(4):
                        qn = qsizes[q]
                        if not qn:
                            continue
                        if SKIP_GATHER:
                            nc.gpsimd.memset(
                                msgs[:, t0 : t0 + qn // 128, :], 0.0)
                        else:
                            nc.gpsimd.dma_gather(
                                out_ap=msgs[:, t0 : t0 + qn // 128, :],
                                in_ap=tquads[:, q * H : (q + 1) * H],
                                idxs_ap=gidxs[:, o16 : o16 + qn // 16],
                                num_idxs=qn, num_idxs_reg=qn, elem_size=H,
                                elem_step=4 * H, single_packet=False)
                        o16 += qn // 16
                        t0 += qn // 128
                    B = bp.tile([128, T * 128], f16, tag="B")
                    nc.vector.tensor_tensor(
                        out=B[:].rearrange("p (t w) -> p t w", w=128),
                        in0=dmods[:, toff : toff + T].unsqueeze(2)
                            .to_broadcast([128, T, 128]),
                        in1=iotas[:].unsqueeze(1).to_broadcast([128, T, 128]),
                        op=mybir.AluOpType.is_equal)
                    # matmuls, grouped per window into one psum tile
                    wps = {}
                    for tl, q, w, st, _sp in tiles:
                        if st:
                            wps[w] = pwin.tile([128, H], f32, tag="win", name=f"win{w}")
                        nc.tensor.matmul(
                            out=wps[w][:],
                            lhsT=B[:, tl * 128 : (tl + 1) * 128],
                            rhs=msgs[:, tl, :],
                            start=st, stop=_sp)
                    for w in sorted(wps):
                        t = tmpp.tile([128, H], f32, tag="wtmp")
                        nc.vector.tensor_tensor(
                            out=t[:], in0=wps[w][:],
                            in1=dinvs[:, w : w + 1].to_broadcast([128, H]),
                            op=mybir.AluOpType.mult)
                        nc.vector.tensor_tensor(
                            out=out16[:, w, :], in0=t[:], in1=g32[:, w, :],
                            op=mybir.AluOpType.add)
                        nc.scalar.activation(out16[:, w, :], out16[:, w, :],
                                             Relu)
                # windows with no edges at all (rare/mini): plain copy
                for w in range(SLOTS):
                    if not any(t[2] == w for ch in chunks for t in ch[3]):
                        nc.vector.tensor_copy(out=out16[:, w, :], in_=g32[:, w, :])
                        nc.scalar.activation(out16[:, w, :], out16[:, w, :],
                                             Relu)

            def transpose_to_featmajor(out16, a16):
                if SKIP_TPOSE:
                    nc.gpsimd.memset(a16[:], 0.0)
                    return
                for s in range(SLOTS):
                    tp = pmisc.tile([H, 128], f16, tag="misc")
                    nc.tensor.transpose(tp[:], out16[:, s, :], idents[:])
                    nc.any.tensor_copy(out=a16[:, s * 128 : (s + 1) * 128],
                                       in_=tp[:])

            def comms(g32):
                """cast to fp16, DMA to DRAM, AllGather -> table AP."""
                g16 = bigp.tile([128, SLOTS, H], f16, tag="g16")
                nc.vector.tensor_copy(
                    out=g16[:].rearrange("p s d -> p (s d)"),
                    in_=g32[:].rearrange("p s d -> p (s d)"))
                gd = dram.tile([NPCP, H], f16, tag="gd")
                nc.sync.dma_start(
                    out=gd[:].rearrange("(s p) d -> p s d", p=128), in_=g16[:])
                tbl = dram.tile([TBL, H], f16, tag="tbl")
                if SKIP_AG:
                    nc.sync.dma_start(out=tbl[: gd.shape[0], :], in_=gd[:])
                else:
                    nc.gpsimd.collective_compute(
                        "AllGather", mybir.AluOpType.bypass,
                        replica_groups=[list(range(NCORES))],
                        ins=[gd.opt()], outs=[tbl.opt()])
                return tbl

            def bail():
                zt = outp.tile([C, NPCP], f32, tag="zt", bufs=1)
                nc.gpsimd.memset(zt[:], 0.0)
                nc.sync.dma_start(out=out_d[:], in_=zt[:])

            # ---------------- layer 1 ----------------
            g32 = bigp.tile([128, SLOTS, H], f32, tag="g32")
            if STOPAT >= 1:
                for s in range(SLOTS):
                    xt = xtp.tile([F, 128], f32, tag="xt")
                    nc.sync.dma_start(out=xt[:], in_=x_t[:, s * 128 : (s + 1) * 128])
                    ps = pmisc.tile([128, H], f32, tag="misc")
                    nc.tensor.matmul(out=ps[:], lhsT=xt[:], rhs=W1s[:],
                                     start=True, stop=True)
                    nc.vector.tensor_tensor(
                        out=g32[:, s, :], in0=ps[:],
                        in1=dinvs[:, s : s + 1].to_broadcast([128, H]),
                        op=mybir.AluOpType.mult)
            else:
                nc.gpsimd.memset(g32[:], 0.0)
            if STOPAT < 2:
                bail()
            else:
                tbl1 = comms(g32)
                if STOPAT < 3:
                    bail()
                else:
                    out16 = bigp.tile([128, SLOTS, H], f16, tag="o16")
                    aggregate(tbl1[:], g32, b1s, out16)
                    a16 = bigp.tile([H, NPCP], f16, tag="a16")
                    transpose_to_featmajor(out16, a16)
                    if STOPAT < 4:
                        bail()
                    else:
                        # ---------------- layer 2 ----------------
                        g32b = bigp.tile([128, SLOTS, H], f32, tag="g32")
                        for s in range(SLOTS):
                            ps = pmisc.tile([128, H], f32, tag="misc")
                            nc.tensor.matmul(out=ps[:],
                                             lhsT=a16[:, s * 128 : (s + 1) * 128],
                                             rhs=W2s[:], start=True, stop=True)
                            nc.vector.tensor_tensor(
                                out=g32b[:, s, :], in0=ps[:],
                                in1=dinvs[:, s : s + 1].to_broadcast([128, H]),
                                op=mybir.AluOpType.mult)
                        tbl2 = comms(g32b)
                        out16b = bigp.tile([128, SLOTS, H], f16, tag="o16")
                        aggregate(tbl2[:], g32b, b2s, out16b)
                        a16b = bigp.tile([H, NPCP], f16, tag="a16b")
                        transpose_to_featmajor(out16b, a16b)

                        # ---------------- head ----------------
                        for j0 in range(0, NPCP, 512):
                            n = min(512, NPCP - j0)
                            ph = pmisc.tile([C, n], f32, tag="misc")
                            nc.tensor.matmul(out=ph[:], lhsT=Wcs[:],
                                             rhs=a16b[:, j0 : j0 + n],
                                             start=True, stop=True)
                            ob = outp.tile([C, n], f32, tag="ob")
                            nc.vector.tensor_scalar(out=ob[:], in0=ph[:],
                                                    scalar1=bcs[:], scalar2=None,
                                                    op0=mybir.AluOpType.add)
                            nc.sync.dma_start(out=out_d[:, j0 : j0 + n], in_=ob[:])

    nc.compile()
    return nc


def _prep_inputs(sched, dinv, gidx_cores, dmod_cores, x, W1, b1, W2, b2, Wc, bc):
    NPC, NPCP, SLOTS = sched["NPC"], sched["NPCP"], sched["SLOTS"]
    H = W1.shape[1]
    in_maps = []
    iota2d = np.tile(np.arange(128, dtype=np.float16), (128, 1))
    ident16 = np.eye(128, dtype=np.float16)
    for c in range(NCORES):
        xs = x[c * NPC : (c + 1) * NPC].astype(np.float32)
        x_t = np.zeros((x.shape[1], NPCP), np.float32)
        x_t[:, :NPC] = xs.T
        dl = dinv[c * NPC : (c + 1) * NPC]
        dnm = np.zeros((SLOTS, 128), np.float32)
        dnm.reshape(-1)[:NPC] = dl
        in_maps.append({
            "x_t": x_t,
            "W1": W1.astype(np.float32),
            "W2h": W2.astype(np.float16),
            "Wch": Wc.astype(np.float16),
            "b1r": np.tile(b1.astype(np.float32), (128, 1)),
            "b2r": np.tile(b2.astype(np.float32), (128, 1)),
            "bcr": bc.astype(np.float32).reshape(-1, 1),
            "dinv_nm": dnm.T.copy(),
            "iota2d": iota2d,
            "ident16": ident16,
            "gidx": gidx_cores[c],
            "dmod": dmod_cores[c],
        })
    return in_maps


_CACHE = {}


def _get_built(edge_index, N, F, H, C):
    key = ("k", N, F, H, C, hash(edge_index.tobytes()))
    if key not in _CACHE:
        sched, dinv, gi, dm = _schedule(edge_index, N)
        nc = _build(sched, F, H, C)
        _CACHE[key] = (sched, dinv, gi, dm, nc)
    return _CACHE[key]


LAST_RESULT = None


def kernel(x, edge_index, W1, b1, W2, b2, Wc, bc):
    global LAST_RESULT
    import os
    from concourse import bass_utils

    x = np.asarray(x)
    edge_index = np.asarray(edge_index)
    N, F = x.shape
    H = W1.shape[1]
    C = Wc.shape[1]
    sched, dinv, gi, dm, nc = _get_built(edge_index, N, F, H, C)
    in_maps = _prep_inputs(sched, dinv, gi, dm, x, W1, b1, W2, b2, Wc, bc)
    trace = os.environ.get("BASS_GCN_TRACE", "0") == "1"
    res = bass_utils.run_bass_kernel_spmd(
        nc, in_maps, core_ids=list(range(NCORES)), trace=trace)
    LAST_RESULT = res
    NPC = sched["NPC"]
    y = np.empty((N, C), np.float32)
    for c in range(NCORES):
        y[c * NPC : (c + 1) * NPC] = res.results[c]["out"][:, :NPC].T
    return y



# revision 3
# speedup vs baseline: 2.1830x; 2.1830x over previous
"""GCN (2x GCNConv + linear head) Trainium2 kernel, 8-core graph-parallel.

Strategy
--------
Nodes are sharded across 8 NeuronCores (dst-range ownership). Per layer:
  h_pre = a_prev @ W          (TensorE, per 128-node tile)
  g     = dinv * h_pre        (fused into PSUM eviction; fp16 copy for comms)
  AllGather g -> full table   (collective, fp16, packed [N,64])
  aggregation: for each edge (s,d): acc[d] += g[s]
      - gather g[s] via dma_gather (GPSIMD SWDGE): table viewed as
        [quads, 256] fp16 (4 nodes / 512B per row, int16 quad indices)
      - segmented reduction via one-hot matmul: per 128-edge tile,
        B[e, j] = (dst_local[e] % 128 == j) built with a DVE is_equal;
        PSUM accumulates all tiles of a 128-dst window. Race-free (unlike
        dma_scatter_add, whose CCE read-modify-write loses updates for
        duplicate indices).
  out = relu(dinv * acc + dinv*g_self + b)   (folded: g'' = dinv*g + b)
  transpose to feature-major for the next layer's matmul (TensorE).
Head: out2 @ Wc + bc per 512-node chunk; host reassembles [N, 40].

The edge schedule (tiles per (window, src%4) cell) is data-dependent but
shared across cores: per-cell tile counts are the max over cores, with
dummy edges (gather idx 0, dst one-hot row of zeros) padding each cell.
"""
import sys

sys.path.insert(0, "/opt/trn_rl_repo")

import numpy as np

NCORES = 8
WCH = 6  # dst windows (128 nodes each) per gather chunk


def _div_up(a, b):
    return -(-a // b)


def _wrap_idx16(idx):
    """Per-call idx layout: element i -> partition i%16, slot i//16."""
    assert len(idx) % 16 == 0
    return idx.astype(np.int16).reshape(-1, 16).T  # [16, S/16]


def _schedule(edge_index, N):
    """Build the shared tile schedule + per-core gather/one-hot data."""
    NPC = N // NCORES
    SLOTS = _div_up(NPC, 128)
    NPCP = SLOTS * 128
    assert NPCP % 4 == 0

    src = np.asarray(edge_index[0], np.int64)
    dst = np.asarray(edge_index[1], np.int64)
    deg = np.bincount(dst, minlength=N).astype(np.float32) + 1.0
    dinv = 1.0 / np.sqrt(deg)

    owner = src // NPC
    sp = owner * NPCP + (src - owner * NPC)  # padded global src index
    quad = (sp // 4).astype(np.int64)
    qq = (sp % 4).astype(np.int64)
    assert quad.max() < 32768, "quad index must fit int16"

    NW = SLOTS  # windows of 128 dst nodes
    core_data = []
    counts = np.zeros((NCORES, NW, 4), np.int64)
    for c in range(NCORES):
        m = (dst >= c * NPC) & (dst < (c + 1) * NPC)
        dl = dst[m] - c * NPC
        w = dl // 128
        key = w * 4 + qq[m]
        order = np.argsort(key, kind="stable")
        core_data.append((quad[m][order], (dl % 128)[order], key[order]))
        np.add.at(counts[c], (w, qq[m]), 1)

    T_cell = _div_up(counts, 128).max(axis=0)  # [NW, 4] shared tiles/cell

    # tile order: chunks of WCH windows; within chunk: q-major (one gather
    # call per q, reading table quad-rows at byte offset 128*q), then w.
    # A window's psum accumulates across its 4 q-cells within one chunk.
    chunks = []  # (S, idx_off16, t_off, [(tl, q, w, start, stop)], qsizes)
    t_off = 0
    idx_off = 0
    for w0 in range(0, NW, WCH):
        ws = range(w0, min(w0 + WCH, NW))
        tiles = []
        first = {w: True for w in ws}
        ntiles_left = {w: int(T_cell[w].sum()) for w in ws}
        qsizes = []
        for q in range(4):
            qn = 0
            for w in ws:
                for _ in range(T_cell[w, q]):
                    st = first[w]
                    first[w] = False
                    ntiles_left[w] -= 1
                    sp = ntiles_left[w] == 0
                    tiles.append((len(tiles), q, w, st, sp))
                    qn += 1
            qsizes.append(qn * 128)
        S = len(tiles) * 128
        chunks.append((S, idx_off // 16, t_off, tiles, qsizes))
        t_off += len(tiles)
        idx_off += S
    T_total = t_off
    S_total = idx_off

    # per-core arrays in tile order
    gidx_cores, dmod_cores = [], []
    for c in range(NCORES):
        cq, cdmod, ckey = core_data[c]
        bounds = np.searchsorted(ckey, np.arange(NW * 4 + 1))
        gidx = np.zeros(S_total, np.int16)
        dmod = np.full(S_total, -1.0, np.float16)
        pos = 0
        for _S, _off16, _toff, tiles, _qs in chunks:
            cell_seen = set()
            for tl, q, w, _st, _sp in tiles:
                k = (w, q)
                if k not in cell_seen:
                    cell_seen.add(k)
                    lo, hi = bounds[w * 4 + q], bounds[w * 4 + q + 1]
                    n = hi - lo
                    cap = T_cell[w, q] * 128
                    gidx[pos : pos + n] = cq[lo:hi]
                    dmod[pos : pos + n] = cdmod[lo:hi].astype(np.float16)
                    pos += cap
        # wrap per gather call (one per nonzero q section of each chunk)
        wrapped = []
        p = 0
        for _S, _off16, _toff, _tiles, qs in chunks:
            for qn in qs:
                if qn:
                    wrapped.append(_wrap_idx16(gidx[p : p + qn]))
                    p += qn
        gw = np.concatenate(wrapped, axis=1)  # [16, S_total/16]
        gidx_cores.append(np.tile(gw, (8, 1)).copy())
        dmod_cores.append(dmod.reshape(T_total, 128).T.copy())  # [128, T]

    sched = dict(
        NPC=NPC, SLOTS=SLOTS, NPCP=NPCP, NW=NW, N=N,
        chunks=chunks, T_total=T_total, S_total=S_total,
    )
    return sched, dinv, gidx_cores, dmod_cores


def _patch_dma_gather_assert():
    """Relax bass's elem_size_bytes %256 assert to %128 for dma_gather.

    The 256B granularity is a transpose-path restriction; the non-transpose
    ucode handles 128B payloads with a 256B-multiple stride (verified exact
    on hardware). Gathering 128B per edge instead of 512B halves DMA time.
    """
    import inspect
    import textwrap
    import concourse.bass as bass

    needle = "elem_size_bytes > 0 and elem_size_bytes % 256 == 0"
    src = inspect.getsource(bass.BassGpSimd.dma_gather)
    if needle not in src:
        return
    src = textwrap.dedent(src.replace(
        needle, "elem_size_bytes > 0 and elem_size_bytes % 128 == 0"))
    ns = {}
    exec(compile(src, "<dma_gather_patched>", "exec"), bass.__dict__, ns)
    bass.BassGpSimd.dma_gather = ns["dma_gather"]


def _build(sched, F, H, C):
    import os
    import concourse.bacc as bacc
    import concourse.mybir as mybir
    import concourse.tile as tile

    SKIP_AG = os.environ.get("SKIP_AG", "0") == "1"
    SKIP_GATHER = os.environ.get("SKIP_GATHER", "0") == "1"
    SKIP_AGG = os.environ.get("SKIP_AGG", "0") == "1"
    SKIP_TPOSE = os.environ.get("SKIP_TPOSE", "0") == "1"
    STOPAT = int(os.environ.get("STOPAT", "99"))

    f32 = mybir.dt.float32
    f16 = mybir.dt.float16
    i16 = mybir.dt.int16
    Relu = mybir.ActivationFunctionType.Relu
    SLOTS, NPCP = sched["SLOTS"], sched["NPCP"]
    S_total, T_total = sched["S_total"], sched["T_total"]
    chunks = sched["chunks"]
    TBL = NCORES * NPCP

    _patch_dma_gather_assert()
    nc = bacc.Bacc("TRN2", target_bir_lowering=False, debug=False,
                   num_devices=NCORES, num_swdge_queues=4)

    x_t = nc.dram_tensor("x_t", [F, NPCP], f32, kind="ExternalInput")
    W1 = nc.dram_tensor("W1", [F, H], f32, kind="ExternalInput")
    W2h = nc.dram_tensor("W2h", [H, H], f16, kind="ExternalInput")
    Wch = nc.dram_tensor("Wch", [H, C], f16, kind="ExternalInput")
    b1in = nc.dram_tensor("b1r", [128, H], f32, kind="ExternalInput")
    b2in = nc.dram_tensor("b2r", [128, H], f32, kind="ExternalInput")
    bcin = nc.dram_tensor("bcr", [C, 1], f32, kind="ExternalInput")
    dinvin = nc.dram_tensor("dinv_nm", [128, SLOTS], f32, kind="ExternalInput")
    iotain = nc.dram_tensor("iota2d", [128, 128], f16, kind="ExternalInput")
    identin = nc.dram_tensor("ident16", [128, 128], f16, kind="ExternalInput")
    gidxin = nc.dram_tensor("gidx", [128, S_total // 16], i16, kind="ExternalInput")
    dmodin = nc.dram_tensor("dmod", [128, T_total], f16, kind="ExternalInput")
    out_d = nc.dram_tensor("out", [C, NPCP], f32, kind="ExternalOutput")

    with tile.TileContext(nc) as tc:
        with (
            tc.tile_pool(name="const", bufs=1) as cp,
            tc.tile_pool(name="big", bufs=1) as bigp,
            tc.tile_pool(name="xt", bufs=3) as xtp,
            tc.tile_pool(name="msgs", bufs=2) as msgp,
            tc.tile_pool(name="bmat", bufs=2) as bp,
            tc.tile_pool(name="tmp", bufs=4) as tmpp,
            tc.tile_pool(name="outsb", bufs=3) as outp,
            tc.tile_pool(name="pwin", bufs=6, space="PSUM") as pwin,
            tc.tile_pool(name="pmisc", bufs=2, space="PSUM") as pmisc,
            tc.tile_pool(name="dram", bufs=4, space="DRAM") as dram,
        ):
            def load_const(dt, ten, shape):
                t = cp.tile(shape, dt, tag=ten.name)
                nc.sync.dma_start(out=t[:], in_=ten[:])
                return t

            W1s = load_const(f32, W1, [F, H])
            W2s = load_const(f16, W2h, [H, H])
            Wcs = load_const(f16, Wch, [H, C])
            b1s = load_const(f32, b1in, [128, H])
            b2s = load_const(f32, b2in, [128, H])
            bcs = load_const(f32, bcin, [C, 1])
            dinvs = load_const(f32, dinvin, [128, SLOTS])
            iotas = load_const(f16, iotain, [128, 128])
            idents = load_const(f16, identin, [128, 128])
            gidxs = load_const(i16, gidxin, [128, S_total // 16])
            dmods = load_const(f16, dmodin, [128, T_total])

            def aggregate(table_ap, g32, bias2d, out16):
                """acc = sum_edges g[src]; out16 = relu-input fp16."""
                # fold: g'' = dinv*g + b  (self-loop + bias), in place
                nc.vector.tensor_tensor(
                    out=g32[:], in0=g32[:],
                    in1=dinvs[:].unsqueeze(2).to_broadcast([128, SLOTS, H]),
                    op=mybir.AluOpType.mult)
                nc.vector.tensor_tensor(
                    out=g32[:], in0=g32[:],
                    in1=bias2d[:].unsqueeze(1).to_broadcast([128, SLOTS, H]),
                    op=mybir.AluOpType.add)
                if SKIP_AGG:
                    for w in range(SLOTS):
                        nc.vector.tensor_copy(out=out16[:, w, :], in_=g32[:, w, :])
                    nc.scalar.activation(
                        out16[:].rearrange("p s d -> p (s d)"),
                        out16[:].rearrange("p s d -> p (s d)"), Relu)
                    return
                tquads = table_ap.rearrange("(a b) d -> a (b d)", b=4)
                for S, off16, toff, tiles, qsizes in chunks:
                    T = S // 128
                    msgs = msgp.tile([128, T, H], f16, tag="msgs")
                    o16 = off16
                    t0 = 0
                    for q in range(4):
                        qn = qsizes[q]
                        if not qn:
                            continue
                        if SKIP_GATHER:
                            nc.gpsimd.memset(
                                msgs[:, t0 : t0 + qn // 128, :], 0.0)
                        else:
                            nc.gpsimd.dma_gather(
                                out_ap=msgs[:, t0 : t0 + qn // 128, :],
                                in_ap=tquads[:, q * H : (q + 1) * H],
                                idxs_ap=gidxs[:, o16 : o16 + qn // 16],
                                num_idxs=qn, num_idxs_reg=qn, elem_size=H,
                                elem_step=4 * H, single_packet=False,
                                queue_num=q)
                        o16 += qn // 16
                        t0 += qn // 128
                    B = bp.tile([128, T * 128], f16, tag="B")
                    nc.vector.tensor_tensor(
                        out=B[:].rearrange("p (t w) -> p t w", w=128),
                        in0=dmods[:, toff : toff + T].unsqueeze(2)
                            .to_broadcast([128, T, 128]),
                        in1=iotas[:].unsqueeze(1).to_broadcast([128, T, 128]),
                        op=mybir.AluOpType.is_equal)
                    # matmuls, grouped per window into one psum tile
                    wps = {}
                    for tl, q, w, st, _sp in tiles:
                        if st:
                            wps[w] = pwin.tile([128, H], f32, tag="win", name=f"win{w}")
                        nc.tensor.matmul(
                            out=wps[w][:],
                            lhsT=B[:, tl * 128 : (tl + 1) * 128],
                            rhs=msgs[:, tl, :],
                            start=st, stop=_sp)
                    for w in sorted(wps):
                        t = tmpp.tile([128, H], f32, tag="wtmp")
                        nc.vector.tensor_tensor(
                            out=t[:], in0=wps[w][:],
                            in1=dinvs[:, w : w + 1].to_broadcast([128, H]),
                            op=mybir.AluOpType.mult)
                        nc.vector.tensor_tensor(
                            out=out16[:, w, :], in0=t[:], in1=g32[:, w, :],
                            op=mybir.AluOpType.add)
                        nc.scalar.activation(out16[:, w, :], out16[:, w, :],
                                             Relu)
                # windows with no edges at all (rare/mini): plain copy
                for w in range(SLOTS):
                    if not any(t[2] == w for ch in chunks for t in ch[3]):
                        nc.vector.tensor_copy(out=out16[:, w, :], in_=g32[:, w, :])
                        nc.scalar.activation(out16[:, w, :], out16[:, w, :],
                                             Relu)

            def transpose_to_featmajor(out16, a16):
                if SKIP_TPOSE:
                    nc.gpsimd.memset(a16[:], 0.0)
                    return
                for s in range(SLOTS):
                    tp = pmisc.tile([H, 128], f16, tag="misc")
                    nc.tensor.transpose(tp[:], out16[:, s, :], idents[:])
                    nc.any.tensor_copy(out=a16[:, s * 128 : (s + 1) * 128],
                                       in_=tp[:])

            def comms(g32):
                """cast to fp16, DMA to DRAM, AllGather -> table AP."""
                g16 = bigp.tile([128, SLOTS, H], f16, tag="g16")
                nc.vector.tensor_copy(
                    out=g16[:].rearrange("p s d -> p (s d)"),
                    in_=g32[:].rearrange("p s d -> p (s d)"))
                gd = dram.tile([NPCP, H], f16, tag="gd")
                nc.sync.dma_start(
                    out=gd[:].rearrange("(s p) d -> p s d", p=128), in_=g16[:])
                tbl = dram.tile([TBL, H], f16, tag="tbl")
                if SKIP_AG:
                    nc.sync.dma_start(out=tbl[: gd.shape[0], :], in_=gd[:])
                else:
                    nc.gpsimd.collective_compute(
                        "AllGather", mybir.AluOpType.bypass,
                        replica_groups=[list(range(NCORES))],
                        ins=[gd.opt()], outs=[tbl.opt()])
                return tbl

            def bail():
                zt = outp.tile([C, NPCP], f32, tag="zt", bufs=1)
                nc.gpsimd.memset(zt[:], 0.0)
                nc.sync.dma_start(out=out_d[:], in_=zt[:])

            # ---------------- layer 1 ----------------
            g32 = bigp.tile([128, SLOTS, H], f32, tag="g32")
            if STOPAT >= 1:
                for s in range(SLOTS):
                    xt = xtp.tile([F, 128], f32, tag="xt")
                    nc.sync.dma_start(out=xt[:], in_=x_t[:, s * 128 : (s + 1) * 128])
                    ps = pmisc.tile([128, H], f32, tag="misc")
                    nc.tensor.matmul(out=ps[:], lhsT=xt[:], rhs=W1s[:],
                                     start=True, stop=True)
                    nc.vector.tensor_tensor(
                        out=g32[:, s, :], in0=ps[:],
                        in1=dinvs[:, s : s + 1].to_broadcast([128, H]),
                        op=mybir.AluOpType.mult)
            else:
                nc.gpsimd.memset(g32[:], 0.0)
            if STOPAT < 2:
                bail()
            else:
                tbl1 = comms(g32)
                if STOPAT < 3:
                    bail()
                else:
                    out16 = bigp.tile([128, SLOTS, H], f16, tag="o16")
                    aggregate(tbl1[:], g32, b1s, out16)
                    a16 = bigp.tile([H, NPCP], f16, tag="a16")
                    transpose_to_featmajor(out16, a16)
                    if STOPAT < 4:
                        bail()
                    else:
                        # ---------------- layer 2 ----------------
                        g32b = bigp.tile([128, SLOTS, H], f32, tag="g32")
                        for s in range(SLOTS):
                            ps = pmisc.tile([128, H], f32, tag="misc")
                            nc.tensor.matmul(out=ps[:],
                                             lhsT=a16[:, s * 128 : (s + 1) * 128],
                                             rhs=W2s[:], start=True, stop=True)
                            nc.vector.tensor_tensor(
                                out=g32b[:, s, :], in0=ps[:],
                                in1=dinvs[:, s : s + 1].to_broadcast([128, H]),
                                op=mybir.AluOpType.mult)
                        tbl2 = comms(g32b)
                        out16b = bigp.tile([128, SLOTS, H], f16, tag="o16")
                        aggregate(tbl2[:], g32b, b2s, out16b)
                        a16b = bigp.tile([H, NPCP], f16, tag="a16b")
                        transpose_to_featmajor(out16b, a16b)

                        # ---------------- head ----------------
                        for j0 in range(0, NPCP, 512):
                            n = min(512, NPCP - j0)
                            ph = pmisc.tile([C, n], f32, tag="misc")
                            nc.tensor.matmul(out=ph[:], lhsT=Wcs[:],
                                             rhs=a16b[:, j0 : j0 + n],
                                             start=True, stop=True)
                            ob = outp.tile([C, n], f32, tag="ob")
                            nc.vector.tensor_scalar(out=ob[:], in0=ph[:],
                                                    scalar1=bcs[:], scalar2=None,
                                                    op0=mybir.AluOpType.add)
                            nc.sync.dma_start(out=out_d[:, j0 : j0 + n], in_=ob[:])

    nc.compile()
    return nc


def _prep_inputs(sched, dinv, gidx_cores, dmod_cores, x, W1, b1, W2, b2, Wc, bc):
    NPC, NPCP, SLOTS = sched["NPC"], sched["NPCP"], sched["SLOTS"]
    H = W1.shape[1]
    in_maps = []
    iota2d = np.tile(np.arange(128, dtype=np.float16), (128, 1))
    ident16 = np.eye(128, dtype=np.float16)
    for c in range(NCORES):
        xs = x[c * NPC : (c + 1) * NPC].astype(np.float32)
        x_t = np.zeros((x.shape[1], NPCP), np.float32)
        x_t[:, :NPC] = xs.T
        dl = dinv[c * NPC : (c + 1) * NPC]
        dnm = np.zeros((SLOTS, 128), np.float32)
        dnm.reshape(-1)[:NPC] = dl
        in_maps.append({
            "x_t": x_t,
            "W1": W1.astype(np.float32),
            "W2h": W2.astype(np.float16),
            "Wch": Wc.astype(np.float16),
            "b1r": np.tile(b1.astype(np.float32), (128, 1)),
            "b2r": np.tile(b2.astype(np.float32), (128, 1)),
            "bcr": bc.astype(np.float32).reshape(-1, 1),
            "dinv_nm": dnm.T.copy(),
            "iota2d": iota2d,
            "ident16": ident16,
            "gidx": gidx_cores[c],
            "dmod": dmod_cores[c],
        })
    return in_maps


_CACHE = {}


def _get_built(edge_index, N, F, H, C):
    key = ("k", N, F, H, C, hash(edge_index.tobytes()))
    if key not in _CACHE:
        sched, dinv, gi, dm = _schedule(edge_index, N)
        nc = _build(sched, F, H, C)
        _CACHE[key] = (sched, dinv, gi, dm, nc)
    return _CACHE[key]


LAST_RESULT = None


def kernel(x, edge_index, W1, b1, W2, b2, Wc, bc):
    global LAST_RESULT
    import os
    from concourse import bass_utils

    x = np.asarray(x)
    edge_index = np.asarray(edge_index)
    N, F = x.shape
    H = W1.shape[1]
    C = Wc.shape[1]
    sched, dinv, gi, dm, nc = _get_built(edge_index, N, F, H, C)
    in_maps = _prep_inputs(sched, dinv, gi, dm, x, W1, b1, W2, b2, Wc, bc)
    trace = os.environ.get("BASS_GCN_TRACE", "0") == "1"
    res = bass_utils.run_bass_kernel_spmd(
        nc, in_maps, core_ids=list(range(NCORES)), trace=trace)
    LAST_RESULT = res
    NPC = sched["NPC"]
    y = np.empty((N, C), np.float32)
    for c in range(NCORES):
        y[c * NPC : (c + 1) * NPC] = res.results[c]["out"][:, :NPC].T
    return y



# revision 7
# speedup vs baseline: 2.4566x; 1.1253x over previous
"""GCN (2x GCNConv + linear head) Trainium2 kernel, 8-core graph-parallel.

Strategy
--------
Nodes are sharded across 8 NeuronCores (dst-range ownership). Per layer:
  h_pre = a_prev @ W          (TensorE, per 128-node tile)
  g     = dinv * h_pre        (fused into PSUM eviction; fp16 copy for comms)
  AllGather g -> full table   (collective, fp16, packed [N,64])
  aggregation: for each edge (s,d): acc[d] += g[s]
      - gather g[s] via dma_gather (GPSIMD SWDGE): table viewed as
        [quads, 256] fp16 (4 nodes / 512B per row, int16 quad indices)
      - segmented reduction via one-hot matmul: per 128-edge tile,
        B[e, j] = (dst_local[e] % 128 == j) built with a DVE is_equal;
        PSUM accumulates all tiles of a 128-dst window. Race-free (unlike
        dma_scatter_add, whose CCE read-modify-write loses updates for
        duplicate indices).
  out = relu(dinv * acc + dinv*g_self + b)   (folded: g'' = dinv*g + b)
  transpose to feature-major for the next layer's matmul (TensorE).
Head: out2 @ Wc + bc per 512-node chunk; host reassembles [N, 40].

The edge schedule (tiles per (window, src%4) cell) is data-dependent but
shared across cores: per-cell tile counts are the max over cores, with
dummy edges (gather idx 0, dst one-hot row of zeros) padding each cell.
"""
import sys

sys.path.insert(0, "/opt/trn_rl_repo")

import numpy as np

NCORES = 8
WCH = 6  # dst windows (128 nodes each) per gather chunk


def _div_up(a, b):
    return -(-a // b)


def _wrap_idx16(idx):
    """Per-call idx layout: element i -> partition i%16, slot i//16."""
    assert len(idx) % 16 == 0
    return idx.astype(np.int16).reshape(-1, 16).T  # [16, S/16]


def _schedule(edge_index, N):
    """Build the shared tile schedule + per-core gather/one-hot data."""
    NPC = N // NCORES
    SLOTS = _div_up(NPC, 128)
    NPCP = SLOTS * 128
    assert NPCP % 4 == 0

    src = np.asarray(edge_index[0], np.int64)
    dst = np.asarray(edge_index[1], np.int64)
    deg = np.bincount(dst, minlength=N).astype(np.float32) + 1.0
    dinv = 1.0 / np.sqrt(deg)

    owner = src // NPC
    sp = owner * NPCP + (src - owner * NPC)  # padded global src index
    quad = (sp // 4).astype(np.int64)
    qq = (sp % 4).astype(np.int64)
    assert quad.max() < 32768, "quad index must fit int16"

    NW = SLOTS  # windows of 128 dst nodes
    core_data = []
    counts = np.zeros((NCORES, NW, 4), np.int64)
    for c in range(NCORES):
        m = (dst >= c * NPC) & (dst < (c + 1) * NPC)
        dl = dst[m] - c * NPC
        w = dl // 128
        key = w * 4 + qq[m]
        order = np.argsort(key, kind="stable")
        core_data.append((quad[m][order], (dl % 128)[order], key[order]))
        np.add.at(counts[c], (w, qq[m]), 1)

    T_cell = _div_up(counts, 128).max(axis=0)  # [NW, 4] shared tiles/cell

    # tile order: chunks of WCH windows; within chunk: q-major (one gather
    # call per q, reading table quad-rows at byte offset 128*q), then w.
    # A window's psum accumulates across its 4 q-cells within one chunk.
    chunks = []  # (S, idx_off16, t_off, [(tl, q, w, start, stop)], qsizes)
    t_off = 0
    idx_off = 0
    for w0 in range(0, NW, WCH):
        ws = range(w0, min(w0 + WCH, NW))
        tiles = []
        first = {w: True for w in ws}
        ntiles_left = {w: int(T_cell[w].sum()) for w in ws}
        qsizes = []
        for q in range(4):
            qn = 0
            for w in ws:
                for _ in range(T_cell[w, q]):
                    st = first[w]
                    first[w] = False
                    ntiles_left[w] -= 1
                    sp = ntiles_left[w] == 0
                    tiles.append((len(tiles), q, w, st, sp))
                    qn += 1
            qsizes.append(qn * 128)
        S = len(tiles) * 128
        chunks.append((S, idx_off // 16, t_off, tiles, qsizes))
        t_off += len(tiles)
        idx_off += S
    T_total = t_off
    S_total = idx_off

    # per-core arrays in tile order
    gidx_cores, dmod_cores = [], []
    for c in range(NCORES):
        cq, cdmod, ckey = core_data[c]
        bounds = np.searchsorted(ckey, np.arange(NW * 4 + 1))
        gidx = np.zeros(S_total, np.int16)
        dmod = np.full(S_total, -1.0, np.float16)
        pos = 0
        for _S, _off16, _toff, tiles, _qs in chunks:
            cell_seen = set()
            for tl, q, w, _st, _sp in tiles:
                k = (w, q)
                if k not in cell_seen:
                    cell_seen.add(k)
                    lo, hi = bounds[w * 4 + q], bounds[w * 4 + q + 1]
                    n = hi - lo
                    cap = T_cell[w, q] * 128
                    gidx[pos : pos + n] = cq[lo:hi]
                    dmod[pos : pos + n] = cdmod[lo:hi].astype(np.float16)
                    pos += cap
        # wrap per gather call (one per nonzero q section of each chunk)
        wrapped = []
        p = 0
        for _S, _off16, _toff, _tiles, qs in chunks:
            for qn in qs:
                if qn:
                    wrapped.append(_wrap_idx16(gidx[p : p + qn]))
                    p += qn
        gw = np.concatenate(wrapped, axis=1)  # [16, S_total/16]
        gidx_cores.append(np.tile(gw, (8, 1)).copy())
        dmod_cores.append(dmod.reshape(T_total, 128).T.copy())  # [128, T]

    sched = dict(
        NPC=NPC, SLOTS=SLOTS, NPCP=NPCP, NW=NW, N=N,
        chunks=chunks, T_total=T_total, S_total=S_total,
    )
    return sched, dinv, gidx_cores, dmod_cores


def _patch_dma_gather_assert():
    """Relax bass's elem_size_bytes %256 assert to %128 for dma_gather.

    The 256B granularity is a transpose-path restriction; the non-transpose
    ucode handles 128B payloads with a 256B-multiple stride (verified exact
    on hardware). Gathering 128B per edge instead of 512B halves DMA time.
    """
    import inspect
    import textwrap
    import concourse.bass as bass

    needle = "elem_size_bytes > 0 and elem_size_bytes % 256 == 0"
    src = inspect.getsource(bass.BassGpSimd.dma_gather)
    if needle not in src:
        return
    src = textwrap.dedent(src.replace(
        needle, "elem_size_bytes > 0 and elem_size_bytes % 128 == 0"))
    ns = {}
    exec(compile(src, "<dma_gather_patched>", "exec"), bass.__dict__, ns)
    bass.BassGpSimd.dma_gather = ns["dma_gather"]


def _build(sched, F, H, C):
    import os
    import concourse.bacc as bacc
    import concourse.mybir as mybir
    import concourse.tile as tile

    SKIP_AG = os.environ.get("SKIP_AG", "0") == "1"
    SKIP_GATHER = os.environ.get("SKIP_GATHER", "0") == "1"
    SKIP_AGG = os.environ.get("SKIP_AGG", "0") == "1"
    SKIP_TPOSE = os.environ.get("SKIP_TPOSE", "0") == "1"
    STOPAT = int(os.environ.get("STOPAT", "99"))

    f32 = mybir.dt.float32
    f16 = mybir.dt.float16
    i16 = mybir.dt.int16
    Relu = mybir.ActivationFunctionType.Relu
    SLOTS, NPCP = sched["SLOTS"], sched["NPCP"]
    S_total, T_total = sched["S_total"], sched["T_total"]
    chunks = sched["chunks"]
    TBL = NCORES * NPCP

    _patch_dma_gather_assert()
    nc = bacc.Bacc("TRN2", target_bir_lowering=False, debug=False,
                   num_devices=NCORES, num_swdge_queues=4)

    x_t = nc.dram_tensor("x_t", [F, NPCP], f32, kind="ExternalInput")
    W1 = nc.dram_tensor("W1", [F, H], f32, kind="ExternalInput")
    W2h = nc.dram_tensor("W2h", [H, H], f16, kind="ExternalInput")
    Wch = nc.dram_tensor("Wch", [H, C], f16, kind="ExternalInput")
    b1in = nc.dram_tensor("b1r", [128, H], f32, kind="ExternalInput")
    b2in = nc.dram_tensor("b2r", [128, H], f32, kind="ExternalInput")
    bcin = nc.dram_tensor("bcr", [C, 1], f32, kind="ExternalInput")
    dinvin = nc.dram_tensor("dinv_nm", [128, SLOTS], f32, kind="ExternalInput")
    iotain = nc.dram_tensor("iota2d", [128, 128], f16, kind="ExternalInput")
    identin = nc.dram_tensor("ident16", [128, 128], f16, kind="ExternalInput")
    gidxin = nc.dram_tensor("gidx", [128, S_total // 16], i16, kind="ExternalInput")
    dmodin = nc.dram_tensor("dmod", [128, T_total], f16, kind="ExternalInput")
    out_d = nc.dram_tensor("out", [C, NPCP], f32, kind="ExternalOutput")

    with tile.TileContext(nc) as tc:
        with (
            tc.tile_pool(name="const", bufs=1) as cp,
            tc.tile_pool(name="big", bufs=1) as bigp,
            tc.tile_pool(name="xt", bufs=3) as xtp,
            tc.tile_pool(name="msgs", bufs=2) as msgp,
            tc.tile_pool(name="bmat", bufs=2) as bp,
            tc.tile_pool(name="tmp", bufs=4) as tmpp,
            tc.tile_pool(name="outsb", bufs=3) as outp,
            tc.tile_pool(name="pwin", bufs=6, space="PSUM") as pwin,
            tc.tile_pool(name="pmisc", bufs=2, space="PSUM") as pmisc,
            tc.tile_pool(name="dram", bufs=4, space="DRAM") as dram,
        ):
            def load_const(dt, ten, shape):
                t = cp.tile(shape, dt, tag=ten.name)
                nc.sync.dma_start(out=t[:], in_=ten[:])
                return t

            W1s = load_const(f32, W1, [F, H])
            W2s = load_const(f16, W2h, [H, H])
            Wcs = load_const(f16, Wch, [H, C])
            b1s = load_const(f32, b1in, [128, H])
            b2s = load_const(f32, b2in, [128, H])
            bcs = load_const(f32, bcin, [C, 1])
            dinvs = load_const(f32, dinvin, [128, SLOTS])
            iotas = load_const(f16, iotain, [128, 128])
            idents = load_const(f16, identin, [128, 128])
            gidxs = load_const(i16, gidxin, [128, S_total // 16])
            dmods = load_const(f16, dmodin, [128, T_total])

            def aggregate(table_ap, g32, bias2d, out16):
                """acc = sum_edges g[src]; out16 = relu-input fp16."""
                # fold: g'' = dinv*g + b  (self-loop + bias), in place
                nc.vector.tensor_tensor(
                    out=g32[:], in0=g32[:],
                    in1=dinvs[:].unsqueeze(2).to_broadcast([128, SLOTS, H]),
                    op=mybir.AluOpType.mult)
                nc.vector.tensor_tensor(
                    out=g32[:], in0=g32[:],
                    in1=bias2d[:].unsqueeze(1).to_broadcast([128, SLOTS, H]),
                    op=mybir.AluOpType.add)
                if SKIP_AGG:
                    for w in range(SLOTS):
                        nc.vector.tensor_copy(out=out16[:, w, :], in_=g32[:, w, :])
                    nc.scalar.activation(
                        out16[:].rearrange("p s d -> p (s d)"),
                        out16[:].rearrange("p s d -> p (s d)"), Relu)
                    return
                tquads = table_ap.rearrange("(a b) d -> a (b d)", b=4)
                for S, off16, toff, tiles, qsizes in chunks:
                    T = S // 128
                    msgs = msgp.tile([128, T, H], f16, tag="msgs")
                    o16 = off16
                    t0 = 0
                    for q in range(4):
                        qn = qsizes[q]
                        if not qn:
                            continue
                        if SKIP_GATHER:
                            nc.gpsimd.memset(
                                msgs[:, t0 : t0 + qn // 128, :], 0.0)
                        else:
                            nc.gpsimd.dma_gather(
                                out_ap=msgs[:, t0 : t0 + qn // 128, :],
                                in_ap=tquads[:, q * H : (q + 1) * H],
                                idxs_ap=gidxs[:, o16 : o16 + qn // 16],
                                num_idxs=qn, num_idxs_reg=qn, elem_size=H,
                                elem_step=4 * H, single_packet=False,
                                queue_num=q)
                        o16 += qn // 16
                        t0 += qn // 128
                    B = bp.tile([128, T * 128], f16, tag="B")
                    nc.vector.tensor_tensor(
                        out=B[:].rearrange("p (t w) -> p t w", w=128),
                        in0=dmods[:, toff : toff + T].unsqueeze(2)
                            .to_broadcast([128, T, 128]),
                        in1=iotas[:].unsqueeze(1).to_broadcast([128, T, 128]),
                        op=mybir.AluOpType.is_equal)
                    # matmuls, grouped per window into one psum tile
                    wps = {}
                    for tl, q, w, st, _sp in tiles:
                        if st:
                            wps[w] = pwin.tile([128, H], f32, tag="win", name=f"win{w}")
                        nc.tensor.matmul(
                            out=wps[w][:],
                            lhsT=B[:, tl * 128 : (tl + 1) * 128],
                            rhs=msgs[:, tl, :],
                            start=st, stop=_sp)
                    for w in sorted(wps):
                        t = tmpp.tile([128, H], f32, tag="wtmp")
                        nc.vector.tensor_tensor(
                            out=t[:], in0=wps[w][:],
                            in1=dinvs[:, w : w + 1].to_broadcast([128, H]),
                            op=mybir.AluOpType.mult)
                        nc.vector.tensor_tensor(
                            out=out16[:, w, :], in0=t[:], in1=g32[:, w, :],
                            op=mybir.AluOpType.add)
                        nc.scalar.activation(out16[:, w, :], out16[:, w, :],
                                             Relu)
                # windows with no edges at all (rare/mini): plain copy
                for w in range(SLOTS):
                    if not any(t[2] == w for ch in chunks for t in ch[3]):
                        nc.vector.tensor_copy(out=out16[:, w, :], in_=g32[:, w, :])
                        nc.scalar.activation(out16[:, w, :], out16[:, w, :],
                                             Relu)

            def transpose_to_featmajor(out16, a16):
                if SKIP_TPOSE:
                    nc.gpsimd.memset(a16[:], 0.0)
                    return
                for s in range(SLOTS):
                    tp = pmisc.tile([H, 128], f16, tag="misc")
                    nc.tensor.transpose(tp[:], out16[:, s, :], idents[:])
                    nc.any.tensor_copy(out=a16[:, s * 128 : (s + 1) * 128],
                                       in_=tp[:])

            def comms(g32):
                """cast to fp16, DMA to DRAM, AllGather -> table AP."""
                g16 = bigp.tile([128, SLOTS, H], f16, tag="g16")
                nc.vector.tensor_copy(
                    out=g16[:].rearrange("p s d -> p (s d)"),
                    in_=g32[:].rearrange("p s d -> p (s d)"))
                gd = dram.tile([NPCP, H], f16, tag="gd")
                nc.sync.dma_start(
                    out=gd[:].rearrange("(s p) d -> p s d", p=128), in_=g16[:])
                tbl = dram.tile([TBL, H], f16, tag="tbl")
                if SKIP_AG:
                    nc.sync.dma_start(out=tbl[: gd.shape[0], :], in_=gd[:])
                else:
                    nc.gpsimd.collective_compute(
                        "AllGather", mybir.AluOpType.bypass,
                        replica_groups=[list(range(NCORES))],
                        ins=[gd.opt()], outs=[tbl.opt()])
                return tbl

            def bail():
                zt = outp.tile([C, NPCP], f32, tag="zt", bufs=1)
                nc.gpsimd.memset(zt[:], 0.0)
                nc.sync.dma_start(out=out_d[:], in_=zt[:])

            # ---------------- layer 1 ----------------
            SLAB = 8  # x_t slots per DMA; GRP slots share one psum bank
            GRP = 4
            g32 = bigp.tile([128, SLOTS, H], f32, tag="g32")
            if STOPAT >= 1:
                for s0 in range(0, SLOTS, SLAB):
                    ns = min(SLAB, SLOTS - s0)
                    xt = xtp.tile([F, SLAB * 128], f32, tag="xt")
                    nc.sync.dma_start(
                        out=xt[:, : ns * 128],
                        in_=x_t[:, s0 * 128 : (s0 + ns) * 128])
                    for g0 in range(0, ns, GRP):
                        ng = min(GRP, ns - g0)
                        ps = pmisc.tile([128, GRP, H], f32, tag="misc")
                        for i in range(ng):
                            nc.tensor.matmul(
                                out=ps[:, i, :],
                                lhsT=xt[:, (g0 + i) * 128 : (g0 + i + 1) * 128],
                                rhs=W1s[:], start=True, stop=True)
                        s = s0 + g0
                        nc.vector.tensor_tensor(
                            out=g32[:, s : s + ng, :], in0=ps[:, :ng, :],
                            in1=dinvs[:, s : s + ng].unsqueeze(2)
                                .to_broadcast([128, ng, H]),
                            op=mybir.AluOpType.mult)
            else:
                nc.gpsimd.memset(g32[:], 0.0)
            if STOPAT < 2:
                bail()
            else:
                tbl1 = comms(g32)
                if STOPAT < 3:
                    bail()
                else:
                    out16 = bigp.tile([128, SLOTS, H], f16, tag="o16")
                    aggregate(tbl1[:], g32, b1s, out16)
                    a16 = bigp.tile([H, NPCP], f16, tag="a16")
                    transpose_to_featmajor(out16, a16)
                    if STOPAT < 4:
                        bail()
                    else:
                        # ---------------- layer 2 ----------------
                        g32b = bigp.tile([128, SLOTS, H], f32, tag="g32")
                        for s0 in range(0, SLOTS, GRP):
                            ng = min(GRP, SLOTS - s0)
                            ps = pmisc.tile([128, GRP, H], f32, tag="misc")
                            for i in range(ng):
                                s = s0 + i
                                nc.tensor.matmul(
                                    out=ps[:, i, :],
                                    lhsT=a16[:, s * 128 : (s + 1) * 128],
                                    rhs=W2s[:], start=True, stop=True)
                            nc.vector.tensor_tensor(
                                out=g32b[:, s0 : s0 + ng, :], in0=ps[:, :ng, :],
                                in1=dinvs[:, s0 : s0 + ng].unsqueeze(2)
                                    .to_broadcast([128, ng, H]),
                                op=mybir.AluOpType.mult)
                        tbl2 = comms(g32b)
                        out16b = bigp.tile([128, SLOTS, H], f16, tag="o16")
                        aggregate(tbl2[:], g32b, b2s, out16b)
                        a16b = bigp.tile([H, NPCP], f16, tag="a16b")
                        transpose_to_featmajor(out16b, a16b)

                        # ---------------- head ----------------
                        for j0 in range(0, NPCP, 512):
                            n = min(512, NPCP - j0)
                            ph = pmisc.tile([C, n], f32, tag="misc")
                            nc.tensor.matmul(out=ph[:], lhsT=Wcs[:],
                                             rhs=a16b[:, j0 : j0 + n],
                                             start=True, stop=True)
                            ob = outp.tile([C, n], f32, tag="ob")
                            nc.vector.tensor_scalar(out=ob[:], in0=ph[:],
                                                    scalar1=bcs[:], scalar2=None,
                                                    op0=mybir.AluOpType.add)
                            nc.sync.dma_start(out=out_d[:, j0 : j0 + n], in_=ob[:])

    nc.compile()
    return nc


def _prep_inputs(sched, dinv, gidx_cores, dmod_cores, x, W1, b1, W2, b2, Wc, bc):
    NPC, NPCP, SLOTS = sched["NPC"], sched["NPCP"], sched["SLOTS"]
    H = W1.shape[1]
    in_maps = []
    iota2d = np.tile(np.arange(128, dtype=np.float16), (128, 1))
    ident16 = np.eye(128, dtype=np.float16)
    for c in range(NCORES):
        xs = x[c * NPC : (c + 1) * NPC].astype(np.float32)
        x_t = np.zeros((x.shape[1], NPCP), np.float32)
        x_t[:, :NPC] = xs.T
        dl = dinv[c * NPC : (c + 1) * NPC]
        dnm = np.zeros((SLOTS, 128), np.float32)
        dnm.reshape(-1)[:NPC] = dl
        in_maps.append({
            "x_t": x_t,
            "W1": W1.astype(np.float32),
            "W2h": W2.astype(np.float16),
            "Wch": Wc.astype(np.float16),
            "b1r": np.tile(b1.astype(np.float32), (128, 1)),
            "b2r": np.tile(b2.astype(np.float32), (128, 1)),
            "bcr": bc.astype(np.float32).reshape(-1, 1),
            "dinv_nm": dnm.T.copy(),
            "iota2d": iota2d,
            "ident16": ident16,
            "gidx": gidx_cores[c],
            "dmod": dmod_cores[c],
        })
    return in_maps


_CACHE = {}


def _get_built(edge_index, N, F, H, C):
    key = ("k", N, F, H, C, hash(edge_index.tobytes()))
    if key not in _CACHE:
        sched, dinv, gi, dm = _schedule(edge_index, N)
        nc = _build(sched, F, H, C)
        _CACHE[key] = (sched, dinv, gi, dm, nc)
    return _CACHE[key]


LAST_RESULT = None


def kernel(x, edge_index, W1, b1, W2, b2, Wc, bc):
    global LAST_RESULT
    import os
    from concourse import bass_utils

    x = np.asarray(x)
    edge_index = np.asarray(edge_index)
    N, F = x.shape
    H = W1.shape[1]
    C = Wc.shape[1]
    sched, dinv, gi, dm, nc = _get_built(edge_index, N, F, H, C)
    in_maps = _prep_inputs(sched, dinv, gi, dm, x, W1, b1, W2, b2, Wc, bc)
    trace = os.environ.get("BASS_GCN_TRACE", "0") == "1"
    res = bass_utils.run_bass_kernel_spmd(
        nc, in_maps, core_ids=list(range(NCORES)), trace=trace)
    LAST_RESULT = res
    NPC = sched["NPC"]
    y = np.empty((N, C), np.float32)
    for c in range(NCORES):
        y[c * NPC : (c + 1) * NPC] = res.results[c]["out"][:, :NPC].T
    return y

